# revision 13
# baseline (speedup 1.0000x reference)
"""Trainium2 Bass kernel for nn_AttentionModel (patch-transformer + MSE loss).

Math (per batch element b of B=32), via the algebraic fold:
    Xr       = [raw patches^T ; ones]             [33, T=1024]
    Xa       = A_b Xr  (per-batch instance-norm as a 33x33 affine A_b)
    scores^T = Xr^T (A_b^T M_qk A_b)^T Xr  in [s, t] layout; exp/16, causal
    pred_u   = (A_b^T M_vo_aug)^T Xr exp(...); row 32 = softmax denom (css)
    loss    += sum((pred_u/css - (rstd*raw_next + shift))^2)

Sharding: data-parallel, 4 batch elements per core x 8 cores; host sums
the per-core partials.

Performance structure (v16, from the v15 ~82us baseline):
  * the instance-norm is folded into the tiny 33x33 matrices (mean/std are
    per-batch scalars: Xa = A_b Xr), so the big x transpose pipeline runs
    on RAW x with no stats dependency -- the stats chain, A_b build and
    N_b/V_b folds (tiny PE matmuls) overlap the transposes.
  * x loads are dispatched as the FIRST instructions (8 split DMAs on the
    two HWDGE queues); each [128,97] f32 chunk is PE-transposed directly
    (ones rows materialize by memsetting staging cols 32/96 to 1.0), no
    regroup DMAs, no separate normalize/cast pass.
  * batch-PAIR packing: batches (A, B) of a pair live at partitions
    0-32 / 64-96; all K=33 / M=33 matmuls issue as two instructions on
    disjoint PE quadrants and run CONCURRENTLY. Concurrent full-partition
    MMs must target different PSUM banks.
  * pu checkerboard (A,h0)->bank0, (B,h0)->bank1, (A,h1)->bank1,
    (B,h1)->bank0; B's t-columns are rotated by 512 (un-rotated in tail).
  * main loop is software-pipelined by one step: pu(j-1) is emitted AFTER
    score(j), so the strict-FIFO PE never head-of-line blocks on exp(j-1);
    exp covers both batches per instruction via 2-bank rect APs.
  * causal diag masks and the tail subtract run on GPSIMD (Pool), which is
    otherwise idle; PSUM evacuations stay on DVE (Pool has no PSUM port).
  * 1/css runs on DVE in a DMA-gathered [128, 8] staging layout; the final
    half uses ScalarE Ln/Exp directly (DMA round-trip can't be hidden).
  * tail: dd = (raw_next*rstd) - pu/css on Pool, then ACT Square with
    per-partition bias=shift accumulates the loss partial.
ScalarE exp (~18.4k causal columns -> ~15.4us minimum) paces the steady
state.
"""

import math

import numpy as np

import concourse.bass as bass
import concourse.mybir as mybir
import concourse.tile as tile
from concourse.bass_utils import run_bass_kernel_spmd
from concourse.masks import make_identity, make_upper_triangular
from concourse.vector_clock import ScopedClock

F32 = mybir.dt.float32
BF16 = mybir.dt.bfloat16
AX = mybir.AxisListType
ALU = mybir.AluOpType
AF = mybir.ActivationFunctionType

N_CORES = 8
B = 32
L = 32768
PS = 32
D = 256
T = L // PS  # 1024
BPC = B // N_CORES  # batch elements per core = 4
NT = T // 128  # 8 s-tiles
KA = PS + 1  # augmented contraction dim (extra ones row)
SCALE = 1.0 / math.sqrt(D)  # 1/16
PB = 64  # partition base of batch B within a pair


class SplitDrainTileContext(tile.TileContext):
    """TileContext whose final drain splits sem waits across multiple drain
    instructions -- this walrus rejects >1 sync wait per instruction."""

    def _drain_and_barrier(self, tick_clock, wait_clock):
        probe = mybir.InstDrain(name=f"I-{self.nc.next_id()}", ins=[], outs=[])
        probe.engine = mybir.EngineType.SP
        wait_clock.add_sem_waits(probe, ScopedClock({None: tick_clock.global_clock}))
        waits = list(probe.sync_info.on_wait) if probe.sync_info else []
        assert self.sems is not None
        handles = {h.num: h for h in self.sems.allocated().values()}
        if not waits:
            self.nc.sync.drain()
        for w in waits:
            d = self.nc.sync.drain()
            d.wait_op(handles[w.id], w.wait_value, "sem-ge", check=False)
        self.nc.all_engine_barrier()
        popped = self.nc._tile_sem_poison_stack.pop()
        assert popped is self._sem_poison
        self.nc.clear_and_free_semaphores(list(self.sems.allocated().values()))
        self.nc.all_engine_barrier()


def split_excess_waits(nc, max_waits=1):
    """This walrus rejects instructions carrying more than one sync wait.
    Hoist extra waits onto the immediately preceding same-engine
    instruction when that instruction signals nothing, else insert a
    wait-only drain."""
    for f in nc.m.functions:
        for blk in f.blocks:
            insts = list(blk.instructions)
            out = []
            prev_by_engine = {}
            changed = False
            for inst in insts:
                si = inst.sync_info
                waits = list(si.on_wait) if si else []
                if len(waits) > max_waits:
                    changed = True
                    extra, keep = waits[:-max_waits], waits[-max_waits:]
                    remaining = []
                    prev = prev_by_engine.get(str(inst.engine))
                    for w in extra:
                        psi = prev.sync_info if prev is not None else None
                        if prev is not None and (
                            psi is None
                            or (len(psi.on_wait) == 0 and len(psi.on_update) == 0)
                        ):
                            prev.sync_info = mybir.SyncInfo(on_wait=[w], on_update=[])
                            prev = None  # one hoist per predecessor
                        else:
                            remaining.append(w)
                    for w in remaining:
                        dr = mybir.InstDrain(name=f"I-{nc.next_id()}", ins=[], outs=[])
                        dr.engine = inst.engine
                        dr.sync_info = mybir.SyncInfo(on_wait=[w], on_update=[])
                        out.append(dr)
                    inst.sync_info = mybir.SyncInfo(
                        on_wait=keep, on_update=list(si.on_update)
                    )
                out.append(inst)
                prev_by_engine[str(inst.engine)] = inst
            if changed:
                blk.instructions = out


def dedupe_ldweights(nc):
    """Drop an InstLdweights whose operand AP is byte-identical to the
    immediately preceding PE instruction's InstLdweights (no other PE
    instruction between them) -- the stationary operand is still loaded.
    Only legal when the elided load carries no sync actions."""
    for f in nc.m.functions:
        for blk in f.blocks:
            insts = list(blk.instructions)
            out = []
            last_pe_ldw_key = None
            changed = False
            for inst in insts:
                if str(inst.engine) != "EngineType.PE":
                    out.append(inst)
                    continue
                tname = type(inst).__name__
                if tname == "InstLdweights":
                    si = inst.sync_info
                    has_sync = si and (len(si.on_wait) or len(si.on_update))
                    try:
                        key = str(inst.ins[0])
                    except Exception:
                        key = None
                    if key is not None and key == last_pe_ldw_key and not has_sync:
                        changed = True
                        continue  # elide duplicate load
                    last_pe_ldw_key = key
                    out.append(inst)
                else:
                    if tname == "InstMatmult":
                        if getattr(inst, "is_transpose", None):
                            last_pe_ldw_key = None
                    else:
                        last_pe_ldw_key = None
                    out.append(inst)
            if changed:
                blk.instructions = out


def build_program():
    nc = bass.Bass("TRN2", target_bir_lowering=False, debug=False, num_devices=N_CORES)

    x_d = nc.dram_tensor("x", [BPC, L], F32, kind="ExternalInput")
    mqk_d = nc.dram_tensor("m_qk", [KA, KA], F32, kind="ExternalInput")
    mvo_d = nc.dram_tensor("m_vo", [KA, KA], F32, kind="ExternalInput")
    out_d = nc.dram_tensor("loss_partial", [1, 1], F32, kind="ExternalOutput")

    from contextlib import ExitStack

    with SplitDrainTileContext(nc) as tc, ExitStack() as ctx:
        cpool = ctx.enter_context(tc.tile_pool(name="consts", bufs=1))
        # PSUM: rotating pool (2x [128,1024] = 4 banks) for transient tiles;
        # persistent pool for pred_u (4 banks).
        prot = ctx.enter_context(tc.tile_pool(name="prot", bufs=2, space="PSUM"))
        ppu = ctx.enter_context(tc.tile_pool(name="ppu", bufs=2, space="PSUM"))
        xpool = ctx.enter_context(tc.tile_pool(name="xc", bufs=2))
        spool = ctx.enter_context(tc.tile_pool(name="small", bufs=8))
        bigpool = ctx.enter_context(tc.tile_pool(name="big", bufs=2))
        epool = ctx.enter_context(tc.tile_pool(name="et", bufs=3))
        scratch = ctx.enter_context(tc.tile_pool(name="scratch", bufs=2))

        # force the ACT table load (Copy/Square/Ln/Exp set) at t=0, ahead
        # of the HWDGE dispatches on the scalar queue -- a late table load
        # stalls the stats chain (self-read: no cross-engine dependency)
        actw = cpool.tile([1, 2], F32)
        nc.scalar.activation(actw[:], actw[:], AF.Exp)

        # ---- x load dispatches FIRST (the front-end is latency-bound) ----
        # zl[p]: [128(u), 8(k), 97] f32; batch A at cols 0-31, ones col 32,
        # zero cols 33-63, batch B at cols 64-95, B-ones col 96.  Partition
        # u, chunk k hold tokens 128k+u; a PE transpose of [:, k, 0:97]
        # yields the augmented pair-packed Xr columns for those tokens.
        CW = KA + PB  # 97 staging cols per chunk
        # direct strided zl loads (1024 x 128B descriptors per batch,
        # descriptor-rate bound) interleaved with small contiguous xc loads
        # that feed the stats chain early; pair 0 first so its transposes
        # and the stats chain run while pair 1 still streams in.
        xc = [xpool.tile([128, L // 128], F32, name=f"xc_{b}") for b in range(BPC)]
        zl2 = [xpool.tile([128, NT * CW], F32, name=f"zl_{p}") for p in range(2)]
        zl = [t[:].rearrange("u (k c) -> u k c", c=CW) for t in zl2]

        def load_zl(p):
            for bi in range(2):
                for h in range(2):
                    b = 2 * p + bi
                    qeng = nc.sync if bi == 0 else nc.scalar
                    qeng.dma_start(
                        zl[p][:, 4 * h : 4 * h + 4, PB * bi : PB * bi + PS],
                        x_d.ap()[b].rearrange("(k u ps) -> u k ps", u=128, ps=PS)[
                            :, 4 * h : 4 * h + 4, :
                        ],
                    )

        def load_xc(b):
            qeng = nc.sync if b % 2 == 0 else nc.scalar
            qeng.dma_start(xc[b][:], x_d.ap()[b].rearrange("(u f) -> u f", u=128))

        load_zl(0)
        load_xc(0)
        load_xc(1)
        load_zl(1)
        load_xc(2)
        load_xc(3)

        # ones / zero bands of the staging tiles (transposed into xnt rows
        # 32/96 and the dead band 33-63); pair 0 first -- its transposes
        # start as soon as its x halves land
        for p in range(2):
            nc.gpsimd.memset(zl[p][:, :, PS : PS + 1], 1.0)
            nc.gpsimd.memset(zl[p][:, :, PB + PS : PB + PS + 1], 1.0)
            nc.gpsimd.memset(zl[p][:, :, PS + 1 : PB], 0.0)

        # ---- constants ----
        ident_f = cpool.tile([128, 128], F32)
        make_identity(nc, ident_f[:])
        # doubled keep-mask (upper incl diag) for the diagonal-block mask of
        # both batches at once
        triu2 = cpool.tile([128, 256], BF16)
        make_upper_triangular(nc, triu2[:, 0:128], val=1.0, diag=True)
        make_upper_triangular(nc, triu2[:, 128:256], val=1.0, diag=True)
        ones_col = cpool.tile([128, 1], F32)
        nc.vector.memset(ones_col[:], 1.0)
        ones_r128 = cpool.tile([1, 128], F32)
        nc.vector.memset(ones_r128[:], 1.0)
        ones_t = cpool.tile([128, PS], BF16)
        nc.vector.memset(ones_t[:], 1.0)

        mqk_sb = cpool.tile([128, KA], F32)
        nc.gpsimd.dma_start(mqk_sb[0:KA, :], mqk_d.ap()[:])
        nc.gpsimd.dma_start(mqk_sb[PB : PB + KA, :], mqk_d.ap()[:])
        mvo_sb = cpool.tile([128, KA], F32)
        nc.gpsimd.dma_start(mvo_sb[0:KA, :], mvo_d.ap()[:])
        nc.gpsimd.dma_start(mvo_sb[PB : PB + KA, :], mvo_d.ap()[:])

        lp_all = cpool.tile([128, 4], F32)  # per-(pair, half) loss partials
        nc.vector.memset(lp_all[:], 0.0)

        # ---- transposes (raw x -> pair-packed augmented Xr) + stats ----
        xnt = [None, None]
        sums = spool.tile([128, 8], F32, tag="sums")  # (s b0..b3, q b0..b3)
        sqtr = [
            scratch.tile([128, NT * PS], BF16, tag="sqt", name=f"sqt_{i}")
            for i in range(2)
        ]
        smtr = [
            scratch.tile([128, NT * PS], BF16, tag="smt", name=f"smt_{i}")
            for i in range(2)
        ]

        def transpose_pair(p):
            xnt[p] = bigpool.tile([128, T], BF16, tag="xnt", name=f"xnt_{p}")
            for h in range(2):
                tp_ps = prot.tile([128, 512], F32, tag="rot", name=f"tp_{p}_{h}")
                for i in range(4):
                    tc = 4 * h + i
                    nc.tensor.transpose(
                        tp_ps[0 : PB + KA, 128 * i : 128 * i + 128],
                        zl[p][:, tc, 0 : PB + KA],
                        ident_f[:],
                    )
                nc.vector.tensor_copy(
                    xnt[p][0 : PB + KA, 512 * h : 512 * h + 512],
                    tp_ps[0 : PB + KA, :],
                )

        def stats_pair(p):
            for bi in range(2):
                b = 2 * p + bi
                rect = xc[b][:].rearrange("u (k ps) -> u k ps", ps=PS)
                # sum on DVE (8ns accumulator readout), square on the
                # (idle pre-exp) ScalarE -- the two run concurrently
                nc.vector.tensor_scalar(
                    out=smtr[bi][:].rearrange("u (k ps) -> u k ps", ps=PS),
                    in0=rect,
                    scalar1=1.0,
                    scalar2=0.0,
                    op0=ALU.mult,
                    op1=ALU.add,
                    accum_out=sums[:, b : b + 1],
                )
                nc.scalar.activation(
                    sqtr[bi][:].rearrange("u (k ps) -> u k ps", ps=PS),
                    rect,
                    AF.Square,
                    accum_out=sums[:, 4 + b : 4 + b + 1],
                )

        transpose_pair(0)
        stats_pair(0)
        transpose_pair(1)
        stats_pair(1)

        # ---- per-batch scalars: mean/rstd/shift, vectorized over batches ----
        tot_ps = ppu.tile([1, 8], F32, tag="pu", name="totps")
        nc.tensor.matmul(
            tot_ps[:], ones_col[:], sums[:], start=True, stop=True,
            skip_group_check=True,
        )
        tot = spool.tile([1, 8], F32, tag="tot")
        nc.vector.tensor_copy(tot[:], tot_ps[:])
        w = spool.tile([1, 12], F32, tag="w")
        scq = spool.tile([1, 8], F32, tag="scq")  # (rstd b0..b3, shift b0..b3)
        # s^2/L in one Square; varn = q - s^2/L; rstd = exp(-ln(varn/(L-1))/2)
        # (dropping the reference's +1e-5 on std: ~1e-5 relative, far below
        # the bf16 noise); shift = -mean*rstd = (s*rstd) * (-1/L)
        nc.scalar.activation(
            w[:, 0:4], tot[:, 0:4], AF.Square, scale=1.0 / math.sqrt(L)
        )
        nc.vector.tensor_tensor(
            out=w[:, 4:8], in0=tot[:, 4:8], in1=w[:, 0:4], op=ALU.subtract
        )
        nc.scalar.activation(w[:, 8:12], w[:, 4:8], AF.Ln, scale=1.0 / (L - 1))
        nc.scalar.activation(scq[:, 0:4], w[:, 8:12], AF.Exp, scale=-0.5)
        nc.vector.tensor_tensor(
            out=w[:, 4:8], in0=tot[:, 0:4], in1=scq[:, 0:4], op=ALU.mult
        )
        nc.scalar.mul(scq[:, 4:8], w[:, 4:8], -1.0 / L)

        # broadcast all 8 scalars to every partition: bc[128, 8]
        bc_ps = ppu.tile([128, 8], F32, tag="pu", name="bcps")
        nc.tensor.matmul(
            bc_ps[:], ones_r128[:], scq[:], start=True, stop=True,
            skip_group_check=True,
        )
        bc = spool.tile([128, 8], F32, tag="bc")
        nc.vector.tensor_copy(bc[:], bc_ps[:])

        # ---- A_b build + N_b / V_b folds (tiny 33x33 PE matmuls) ----
        mqkb = [None, None]
        mvob = [None, None]

        def fold_pair(p):
            ab = spool.tile([128, KA], F32, tag="ab", name=f"ab_{p}")
            for bi in range(2):
                r0 = PB * bi
                nc.vector.tensor_scalar(
                    out=ab[r0 : r0 + KA, 0:PS],
                    in0=ident_f[r0 : r0 + KA, r0 : r0 + PS],
                    scalar1=bc[r0 : r0 + KA, 2 * p + bi : 2 * p + bi + 1],
                    scalar2=None,
                    op0=ALU.mult,
                )
                nc.vector.tensor_copy(
                    ab[r0 : r0 + PS, PS : PS + 1],
                    bc[r0 : r0 + PS, 4 + 2 * p + bi : 5 + 2 * p + bi],
                )
                nc.vector.memset(ab[r0 + PS : r0 + KA, PS : PS + 1], 1.0)
            # R = M_qk^T A ; N_b = R^T A = A^T M_qk A ; V_b = A^T M_vo
            r_ps = prot.tile([128, KA], F32, tag="rot", name=f"rps_{p}")
            for bi in range(2):
                r0 = PB * bi
                nc.tensor.matmul(
                    r_ps[r0 : r0 + KA, :], mqk_sb[r0 : r0 + KA, :],
                    ab[r0 : r0 + KA, :], start=True, stop=True,
                    skip_group_check=True,
                )
            rs = spool.tile([128, KA], F32, tag="rs", name=f"rs_{p}")
            nc.vector.tensor_copy(rs[0 : PB + KA, :], r_ps[0 : PB + KA, :])
            nb_ps = prot.tile([128, 2 * KA], F32, tag="rot", name=f"nbps_{p}")
            for bi in range(2):
                r0 = PB * bi
                nc.tensor.matmul(
                    nb_ps[r0 : r0 + KA, 0:KA], rs[r0 : r0 + KA, :],
                    ab[r0 : r0 + KA, :], start=True, stop=True,
                    skip_group_check=True,
                )
                nc.tensor.matmul(
                    nb_ps[r0 : r0 + KA, KA : 2 * KA], ab[r0 : r0 + KA, :],
                    mvo_sb[r0 : r0 + KA, :], start=True, stop=True,
                    skip_group_check=True,
                )
            mqkb[p] = spool.tile([128, KA], BF16, tag="mqkb", name=f"mqkb_{p}")
            mvob[p] = spool.tile([128, KA], BF16, tag="mvob", name=f"mvob_{p}")
            nc.vector.tensor_copy(
                mqkb[p][0 : PB + KA, :], nb_ps[0 : PB + KA, 0:KA]
            )
            nc.vector.tensor_copy(
                mvob[p][0 : PB + KA, :], nb_ps[0 : PB + KA, KA : 2 * KA]
            )

        fold_pair(0)
        # rcol[p]: col0 = rstd, col1 = shift on the batch's own partitions
        # (A rows 0-31, B rows 64-95), zero elsewhere (Square bias reads 0-95)
        rcol = []
        for p in range(2):
            rc = spool.tile([128, 2], F32, tag="rcol", name=f"rcol_{p}")
            nc.vector.memset(rc[PS:PB, :], 0.0)
            nc.vector.tensor_copy(
                rc[0:PS, :],
                bc[0:PS, :].rearrange("u (g b) -> u b g", g=2)[:, 2 * p, :],
            )
            nc.vector.tensor_copy(
                rc[PB : PB + PS, :],
                bc[PB : PB + PS, :].rearrange("u (g b) -> u b g", g=2)[
                    :, 2 * p + 1, :
                ],
            )
            rcol.append(rc)


        # ---- per-pair state ----
        y = [None, None]
        vw = [None, None]
        pu = [None, None]
        rr = [None, None]
        bcrs = [None, None]
        predt = [None, None]
        dds = [None, None]

        def prologue_alloc(p):
            pu[p] = ppu.tile([128, 1024], F32, tag="pu", name=f"pu_{p}")
            rr[p] = scratch.tile([128, 1024], BF16, tag="rr", name=f"rr_{p}")
            bcrs[p] = scratch.tile([128, 1024], F32, tag="bcr", name=f"bcr_{p}")
            predt[p] = scratch.tile([128, 1024], BF16, tag="predt", name=f"predt_{p}")
            dds[p] = scratch.tile([128, 1024], BF16, tag="dd", name=f"dd_{p}")
            nc.gpsimd.memset(dds[p][PS:PB, 0 : T - 1], 0.0)

        def y_half(p, n):
            # Y = N_b^T Xr for one t-half, pair-concurrent
            xt = xnt[p]
            if y[p] is None:
                y[p] = bigpool.tile([128, T], BF16, tag="y", name=f"y_{p}")
            y_ps = prot.tile([128, 512], F32, tag="rot", name=f"yps_{p}_{n}")
            nc.tensor.matmul(
                y_ps[0:KA, :], mqkb[p][0:KA, :],
                xt[0:KA, n * 512 : (n + 1) * 512],
                start=True, stop=True, skip_group_check=True,
            )
            nc.tensor.matmul(
                y_ps[PB : PB + KA, :], mqkb[p][PB : PB + KA, :],
                xt[PB : PB + KA, n * 512 : (n + 1) * 512],
                start=True, stop=True, skip_group_check=True,
            )
            nc.vector.tensor_copy(
                y[p][0 : PB + KA, n * 512 : (n + 1) * 512], y_ps[0 : PB + KA, :]
            )

        def vw_all(p):
            # VW = Xr^T V_b: A_j in bank0 at 64j, B_j in bank1 (two
            # concurrent full-partition MMs must not share a PSUM bank)
            xt = xnt[p]
            vw_ps = prot.tile([128, 1024], F32, tag="rot", name=f"vwps_{p}")
            for j in range(NT):
                nc.tensor.matmul(
                    vw_ps[:, 64 * j : 64 * j + KA],
                    xt[0:KA, j * 128 : (j + 1) * 128],
                    mvob[p][0:KA, :],
                    start=True, stop=True, skip_group_check=True,
                )
                nc.tensor.matmul(
                    vw_ps[:, 512 + 64 * j : 512 + 64 * j + KA],
                    xt[PB : PB + KA, j * 128 : (j + 1) * 128],
                    mvob[p][PB : PB + KA, :],
                    start=True, stop=True, skip_group_check=True,
                )
            # vw cols: A_j at 33j, B_j at 264+33j
            vw[p] = bigpool.tile([128, NT * 2 * KA], BF16, tag="vw", name=f"vw_{p}")
            nc.vector.tensor_copy(
                vw[p][:].rearrange("u (s e) -> u s e", e=KA),
                vw_ps[:].rearrange("u (s e) -> u s e", e=64)[:, :, 0:KA],
            )

        def main_half(p, n):
            """scores -> exp -> PV for one t-half, pair-concurrent,
            software-pipelined: pu(j-1) is emitted after score(j) so the
            strict-FIFO PE never waits on exp(j-1) before score(j).
            pu checkerboard: (A,h) -> bank h, (B,h) -> bank 1-h."""
            xt, yp, vwp, pup = xnt[p], y[p], vw[p], pu[p]
            nj = 4 * n + 4
            bcol = (1 - n) * 512

            def emit_pu(j, off):
                nc.tensor.matmul(
                    pup[0:KA, n * 512 + off : (n + 1) * 512],
                    vwp[:, j * KA : (j + 1) * KA],
                    ets[j][:, off:512],
                    start=(j == 0), stop=(j == nj - 1), skip_group_check=True,
                )
                nc.tensor.matmul(
                    pup[PB : PB + KA, bcol + off : bcol + 512],
                    vwp[:, NT * KA + j * KA : NT * KA + (j + 1) * KA],
                    ets[j][:, 512 + off : 1024],
                    start=(j == 0), stop=(j == nj - 1), skip_group_check=True,
                )

            ets = {}
            offs = {}
            prev = None
            for j in range(nj):
                off = max(0, j * 128 - n * 512)
                offs[j] = off
                diag = j * 128 >= n * 512
                sc_ps = prot.tile(
                    [128, 1024], F32, tag="rot", name=f"scps_{p}_{n}_{j}"
                )
                nc.tensor.matmul(
                    sc_ps[:, off:512],
                    xt[0:KA, j * 128 : (j + 1) * 128],
                    yp[0:KA, n * 512 + off : (n + 1) * 512],
                    start=True, stop=True, skip_group_check=True,
                )
                nc.tensor.matmul(
                    sc_ps[:, 512 + off : 1024],
                    xt[PB : PB + KA, j * 128 : (j + 1) * 128],
                    yp[PB : PB + KA, n * 512 + off : (n + 1) * 512],
                    start=True, stop=True, skip_group_check=True,
                )
                et = epool.tile([128, 1024], BF16, tag="et", name=f"et_{p}_{n}_{j}")
                ets[j] = et
                nc.scalar.activation(
                    et[:].rearrange("u (b c) -> u b c", b=2)[:, :, off:512],
                    sc_ps[:].rearrange("u (b c) -> u b c", b=2)[:, :, off:512],
                    AF.Exp,
                    scale=SCALE,
                )
                if diag:
                    # zero the s > t half of the diagonal block (Pool is idle
                    # mid-stream; keeps DVE and the PE free)
                    db = et[:].rearrange("u (b c) -> u b c", b=2)[
                        :, :, off : off + 128
                    ]
                    nc.vector.tensor_tensor(
                        out=db, in0=db,
                        in1=triu2[:].rearrange("u (b c) -> u b c", b=2),
                        op=ALU.mult,
                    )
                if prev is not None:
                    emit_pu(prev, offs[prev])
                prev = j
            emit_pu(prev, offs[prev])

        def epiA(p, n):
            """1/colsum for one half: css rows (32, A-cols) and (96, B-cols)
            DMA-gathered to a [128, 8] layout, DVE reciprocal, DMA back."""
            pup = pu[p]
            bcol = (1 - n) * 512
            csb = scratch.tile([128, 1024], F32, tag="lnr", name=f"csb_{p}_{n}")
            nc.vector.tensor_copy(csb[0 : PB + PS + 1, :], pup[0 : PB + PS + 1, :])
            stg = spool.tile([128, 8], F32, tag="stg", name=f"stg_{p}_{n}")
            nc.sync.dma_start(
                stg[:, 0:4].rearrange("p q -> p () q"),
                csb[PS : PS + 1, n * 512 : (n + 1) * 512].rearrange(
                    "p (a q) -> p a q", q=4
                ),
            )
            nc.scalar.dma_start(
                stg[:, 4:8].rearrange("p q -> p () q"),
                csb[PB + PS : PB + PS + 1, bcol : bcol + 512].rearrange(
                    "p (a q) -> p a q", q=4
                ),
            )
            rstg = spool.tile([128, 8], F32, tag="rstg", name=f"rstg_{p}_{n}")
            nc.vector.reciprocal(rstg[:], stg[:])
            rb16 = spool.tile([128, 8], BF16, tag="rb16", name=f"rb16_{p}_{n}")
            nc.vector.tensor_copy(rb16[:], rstg[:])
            rrp = rr[p]
            nc.sync.dma_start(
                rrp[PS : PS + 1, n * 512 : (n + 1) * 512].rearrange(
                    "p (a q) -> p a q", q=4
                ),
                rb16[:, 0:4].rearrange("p q -> p () q"),
            )
            nc.scalar.dma_start(
                rrp[PB + PS : PB + PS + 1, bcol : bcol + 512].rearrange(
                    "p (a q) -> p a q", q=4
                ),
                rb16[:, 4:8].rearrange("p q -> p () q"),
            )
        def epiA_scalar(p, n):
            """Same as epiA but via ScalarE Ln/Exp straight from PSUM --
            no DMA round-trip latency; used for the final half where the
            DMA latency cannot be hidden."""
            pup = pu[p]
            bcol = (1 - n) * 512
            rrp = rr[p]
            lnr = scratch.tile([128, 1024], F32, tag="lnr", name=f"lnr_{p}_{n}")
            nc.scalar.activation(
                lnr[0 : PS + 1, n * 512 : (n + 1) * 512],
                pup[0 : PS + 1, n * 512 : (n + 1) * 512],
                AF.Ln,
            )
            nc.scalar.activation(
                rrp[0 : PS + 1, n * 512 : (n + 1) * 512],
                lnr[0 : PS + 1, n * 512 : (n + 1) * 512],
                AF.Exp, scale=-1.0,
            )
            nc.scalar.activation(
                lnr[0 : PB + PS + 1, bcol : bcol + 512],
                pup[0 : PB + PS + 1, bcol : bcol + 512],
                AF.Ln,
            )
            nc.scalar.activation(
                rrp[0 : PB + PS + 1, bcol : bcol + 512],
                lnr[0 : PB + PS + 1, bcol : bcol + 512],
                AF.Exp, scale=-1.0,
            )

        def epiB(p, n):
            """PE broadcast of 1/css + evacuation to sbuf (emitted late so
            the PE FIFO never blocks on the recip DMA chain)."""
            bcol = (1 - n) * 512
            rrp = rr[p]
            bcr_ps = prot.tile([128, 1024], F32, tag="rot", name=f"bcrps_{p}_{n}")
            nc.tensor.matmul(
                bcr_ps[0:PS, n * 512 : (n + 1) * 512],
                ones_t[PS : PS + 1, :],
                rrp[PS : PS + 1, n * 512 : (n + 1) * 512],
                start=True, stop=True, skip_group_check=True,
            )
            nc.tensor.matmul(
                bcr_ps[PB : PB + PS, bcol : bcol + 512],
                ones_t[PB + PS : PB + PS + 1, :],
                rrp[PB + PS : PB + PS + 1, bcol : bcol + 512],
                start=True, stop=True, skip_group_check=True,
                tile_position=(PB + PS, PB),
            )
            nc.vector.tensor_copy(
                bcrs[p][0:PS, n * 512 : (n + 1) * 512],
                bcr_ps[0:PS, n * 512 : (n + 1) * 512],
            )
            nc.vector.tensor_copy(
                bcrs[p][PB : PB + PS, bcol : bcol + 512],
                bcr_ps[PB : PB + PS, bcol : bcol + 512],
            )

        def tail_half(p, n):
            """pred = pu/css; dd = (raw_next*rstd) - pred (Pool, target norm
            folded); ACT Square with bias=shift accumulates the partial.
            dd col c holds t=c; the B rows read checkerboarded pred cols."""
            pup, xt, pt, dd = pu[p], xnt[p], predt[p], dds[p]
            rc = rcol[p]
            bcol = (1 - n) * 512
            nc.vector.tensor_tensor(
                out=pt[0:PS, n * 512 : (n + 1) * 512],
                in0=pup[0:PS, n * 512 : (n + 1) * 512],
                in1=bcrs[p][0:PS, n * 512 : (n + 1) * 512],
                op=ALU.mult,
            )
            nc.vector.tensor_tensor(
                out=pt[PB : PB + PS, bcol : bcol + 512],
                in0=pup[PB : PB + PS, bcol : bcol + 512],
                in1=bcrs[p][PB : PB + PS, bcol : bcol + 512],
                op=ALU.mult,
            )
            # dd cols for this half: t in [512n, 512n+512) (clip t=1023)
            c0 = n * 512
            c1 = min((n + 1) * 512, T - 1)
            nc.vector.scalar_tensor_tensor(
                out=dd[0:PS, c0:c1],
                in0=xt[0:PS, c0 + 1 : c1 + 1],
                scalar=rc[0:PS, 0:1],
                in1=pt[0:PS, c0:c1],
                op0=ALU.mult,
                op1=ALU.subtract,
            )
            nc.vector.scalar_tensor_tensor(
                out=dd[PB : PB + PS, c0:c1],
                in0=xt[PB : PB + PS, c0 + 1 : c1 + 1],
                scalar=rc[PB : PB + PS, 0:1],
                in1=pt[PB : PB + PS, bcol : bcol + (c1 - c0)],
                op0=ALU.mult,
                op1=ALU.subtract,
            )
            # dd = raw*rstd - pred = -(pred - target) + shift;
            # Square(dd*1 + shift) accumulates (pred - target)^2
            nc.scalar.activation(
                dds[p][0 : PB + PS, c0:c1],
                dds[p][0 : PB + PS, c0:c1],
                AF.Square,
                bias=rc[0 : PB + PS, 1:2],
                accum_out=lp_all[0 : PB + PS, 2 * p + n : 2 * p + n + 1],
            )

        # ---- software-pipelined emission ----
        prologue_alloc(0)
        y_half(0, 0)
        vw_all(0)
        prologue_alloc(1)
        main_half(0, 0)
        fold_pair(1)
        y_half(0, 1)
        epiA(0, 0)
        y_half(1, 0)
        vw_all(1)
        main_half(0, 1)
        y_half(1, 1)
        epiA(0, 1)
        epiB(0, 0)
        tail_half(0, 0)
        main_half(1, 0)
        epiA(1, 0)
        epiB(0, 1)
        tail_half(0, 1)
        main_half(1, 1)
        epiA_scalar(1, 1)
        epiB(1, 0)
        tail_half(1, 0)
        epiB(1, 1)
        tail_half(1, 1)

        # ---- final: total partial over pairs & partitions ----
        lsum = spool.tile([128, 1], F32)
        nc.vector.reduce_sum(lsum[:], lp_all[:], axis=AX.X)
        tot_ps2 = prot.tile([1, 1], F32, tag="rot")
        nc.tensor.matmul(tot_ps2[:], ones_col[:], lsum[:], start=True, stop=True)
        out_sb = spool.tile([1, 1], F32)
        nc.vector.tensor_copy(out_sb[:], tot_ps2[:])
        nc.gpsimd.dma_start(out_d.ap()[:], out_sb[:])

    split_excess_waits(nc)
    dedupe_ldweights(nc)
    return nc


_program_cache = {}


def _get_program():
    if "nc" not in _program_cache:
        _program_cache["nc"] = build_program()
    return _program_cache["nc"]


def make_in_maps(x, W_proj, b_proj, W_qkv, b_qkv, W_out, b_out, W_head, b_head):
    f8 = np.float64
    w_eff = W_proj.astype(f8) @ W_qkv.astype(f8)  # [32, 768]
    b_eff = b_proj.astype(f8) @ W_qkv.astype(f8) + b_qkv.astype(f8)  # [768]
    w_aug = np.concatenate([w_eff, b_eff[None, :]], axis=0)  # [33, 768]
    wq, wk, wv = w_aug[:, 0:D], w_aug[:, D : 2 * D], w_aug[:, 2 * D : 3 * D]
    m_qk = wq @ wk.T  # [33, 33]
    w_oh = W_out.astype(f8) @ W_head.astype(f8)  # [256, 32]
    b_oh = b_out.astype(f8) @ W_head.astype(f8) + b_head.astype(f8)  # [32]
    m_vo = wv @ w_oh  # [33, 32]
    m_vo[PS, :] += b_oh
    e_ones = np.zeros((KA, 1), f8)
    e_ones[PS, 0] = 1.0  # selects Xr's ones row -> colsum output column
    m_vo_aug = np.concatenate([m_vo, e_ones], axis=1)  # [33, 33]

    mqk_f = np.ascontiguousarray(m_qk.astype(np.float32))
    mvo_f = np.ascontiguousarray(m_vo_aug.astype(np.float32))

    in_maps = []
    for core in range(N_CORES):
        xs = np.ascontiguousarray(x[core * BPC : (core + 1) * BPC])
        in_maps.append({"x": xs, "m_qk": mqk_f, "m_vo": mvo_f})
    return in_maps


def kernel(**inputs) -> np.ndarray:
    inputs = {k: np.asarray(v) for k, v in inputs.items()}
    nc = _get_program()
    in_maps = make_in_maps(**inputs)
    res = run_bass_kernel_spmd(nc, in_maps, core_ids=list(range(N_CORES)))
    total = sum(float(res.results[i]["loss_partial"][0, 0]) for i in range(N_CORES))
    loss = total / (B * (T - 1) * PS)
    return np.float32(loss)


if __name__ == "__main__":
    rng = np.random.default_rng(0)
    ins = {
        "x": rng.standard_normal((B, L)).astype(np.float32),
        "W_proj": (rng.standard_normal((PS, D)) / math.sqrt(PS)).astype(np.float32),
        "b_proj": np.zeros(D, np.float32),
        "W_qkv": (rng.standard_normal((D, 3 * D)) / math.sqrt(D)).astype(np.float32),
        "b_qkv": np.zeros(3 * D, np.float32),
        "W_out": (rng.standard_normal((D, D)) / math.sqrt(D)).astype(np.float32),
        "b_out": np.zeros(D, np.float32),
        "W_head": (rng.standard_normal((D, PS)) / math.sqrt(D)).astype(np.float32),
        "b_head": np.zeros(PS, np.float32),
    }
    got = kernel(**ins)
    print("kernel loss:", got)
